# revision 2
# baseline (speedup 1.0000x reference)
"""Trainium2 Bass kernel for nn_Encoding (VQ codebook soft-assignment encoding).

Reference computation (per batch b, with n = H*W pixels):
    xr[n, d]   = x[b].reshape(D, N).T
    sl[n, k]   = scale_k^2 * (||xr_n||^2 - 2 xr_n.c_k + ||c_k||^2)
    a[n, k]    = softmax_k(sl)
    e[b, k, d] = sum_n a[n,k] * xr[n,d]  -  (sum_n a[n,k]) * c[k,d]

Sharding: data-parallel over batch: 16 batches -> 8 cores x 2 batches each.
Codewords/scale replicated; no collectives.

Device mapping per core (B_PER_CORE=2, D=512, N=4096, K=32), processed in
groups of 1024 pixels (8 subtiles of 128 pixels):
  - x arrives [d, n] f32; DVE casts to bf16 once (2x mode).
  - mm1 runs with the x-tile as the STATIONARY operand (bf16 -> fast weight
    load) and the tiny codebook as moving: psum_lin[n128, k] += xh[d128,
    n128].T @ cbf[d128, k] accumulated over 4 d-chunks.  This lands the
    logits directly in [pixel, k] layout -- no logit transpose needed.
  - softmax shortcut (validated to 4e-8 frobenius vs exact): instead of a
    per-pixel max, subtract the bound x2_n*s2max:
        es = -2 s2_k (x.c_k) + x2_n (s2_k - s2max)   in [-900, ~1]
    The s2_k c2_k term is dropped entirely (|c2*s2| <= 1e-2, contributes
    ~2e-9 relative error).  x2 is precomputed on the host and shipped
    pre-transposed to pixel-major layout so it DMAs as contiguous lines.
  - x bf16 tiles are PE-transposed [128, 128] into psum, copied to SBUF
    (split across ACT and DVE), then mm2 contracts n:
    psum_e[k, d] += a[n128, k].T @ xt[n128, d512]; asum via a ones matmul
    in a parallel psum bank.
  - e = psum_e - asum[k]*c[k, d] on DVE, then DMA out.
"""

import numpy as np

import concourse.bass as bass
import concourse.bacc as bacc
import concourse.mybir as mybir
from concourse import tile

F32 = mybir.dt.float32
BF16 = mybir.dt.bfloat16
AF = mybir.ActivationFunctionType
AX = mybir.AxisListType
ALU = mybir.AluOpType

B, D, H, W, K = 16, 512, 64, 64, 32
N = H * W                    # 4096 pixels per batch
NCORES = 8
BPC = B // NCORES            # 2 batches per core
DC = D // 128                # 4 contraction chunks
NG = N // 1024               # 4 pixel-groups of 1024 per batch
NSUB = 8                     # 128-pixel subtiles per group
NS = N // 128                # 32 subtiles per batch


def build_nc() -> bass.Bass:
    nc = bacc.Bacc("TRN2", target_bir_lowering=False, debug=False,
                   num_devices=NCORES)

    x = nc.dram_tensor("x", [BPC, D, N], F32, kind="ExternalInput").ap()
    cbf = nc.dram_tensor("cbf", [128, DC, K], BF16, kind="ExternalInput").ap()
    c_kd = nc.dram_tensor("c_kd", [K, D], F32, kind="ExternalInput").ap()
    s2drep = nc.dram_tensor("s2drep", [128, K], F32, kind="ExternalInput").ap()
    x2sT = nc.dram_tensor("x2sT", [BPC, 128, NS], F32, kind="ExternalInput").ap()
    ones_bf = nc.dram_tensor("ones_bf", [128, 2], BF16, kind="ExternalInput").ap()
    ident_bf = nc.dram_tensor("ident_bf", [128, 128], BF16, kind="ExternalInput").ap()
    e = nc.dram_tensor("e", [BPC, K, D], F32, kind="ExternalOutput").ap()

    from contextlib import ExitStack
    with tile.TileContext(nc) as tc, ExitStack() as ctx:
        const = ctx.enter_context(tc.tile_pool(name="const", bufs=1))
        xpool = ctx.enter_context(tc.tile_pool(name="x", bufs=3))
        xhpool = ctx.enter_context(tc.tile_pool(name="xh", bufs=2))
        xtpool = ctx.enter_context(tc.tile_pool(name="xt", bufs=6))
        smpool = ctx.enter_context(tc.tile_pool(name="softmax", bufs=2))
        outpool = ctx.enter_context(tc.tile_pool(name="out", bufs=2))
        ps_lin = ctx.enter_context(tc.tile_pool(name="ps_lin", bufs=2, space="PSUM"))
        ps_xt = ctx.enter_context(tc.tile_pool(name="ps_xt", bufs=2, space="PSUM"))
        ps_e = ctx.enter_context(tc.tile_pool(name="ps_e", bufs=1, space="PSUM"))
        ps_as = ctx.enter_context(tc.tile_pool(name="ps_as", bufs=1, space="PSUM"))

        # Constants, loaded once.
        cbf_sb = const.tile([128, DC, K], BF16)
        nc.sync.dma_start(out=cbf_sb[:], in_=cbf[:])
        ckd_sb = const.tile([K, D], F32)
        nc.sync.dma_start(out=ckd_sb[:], in_=c_kd[:])
        s2d_sb = const.tile([128, K], F32)
        nc.sync.dma_start(out=s2d_sb[:], in_=s2drep[:])
        x2_sb = const.tile([128, BPC, NS], F32)
        for b in range(BPC):
            nc.sync.dma_start(out=x2_sb[:, b, :], in_=x2sT[b])
        onbf_sb = const.tile([128, 2], BF16)
        nc.sync.dma_start(out=onbf_sb[:], in_=ones_bf[:])
        idbf_sb = const.tile([128, 128], BF16)
        nc.sync.dma_start(out=idbf_sb[:], in_=ident_bf[:])

        for b in range(BPC):
            psum_e = ps_e.tile([K, D], F32)
            psum_as = ps_as.tile([K, 2], F32)
            for g in range(NG):
                n0 = g * 1024

                # ---- load one n-group of x: [128, DC, 1024] ([d, n]) ----
                xg = xpool.tile([128, DC, 1024], F32, tag="xg")
                for c in range(DC):
                    nc.sync.dma_start(
                        out=xg[:, c, :],
                        in_=x[b, c * 128:(c + 1) * 128, n0:n0 + 1024])

                # ---- cast to bf16 (DVE 2x), split for pipelining ----
                xh = xhpool.tile([128, DC, 1024], BF16, tag="xh")
                nc.vector.tensor_copy(xh[:, 0:2, :], xg[:, 0:2, :])
                nc.vector.tensor_copy(xh[:, 2:4, :], xg[:, 2:4, :])

                # ---- mm1: logits directly in [pixel, k] layout ----
                psum_lin = ps_lin.tile([128, NSUB, K], F32)
                for j in range(NSUB):
                    js = slice(j * 128, (j + 1) * 128)
                    for c in range(DC):
                        nc.tensor.matmul(
                            psum_lin[:, j, :], lhsT=xh[:, c, js],
                            rhs=cbf_sb[:, c, :],
                            start=(c == 0), stop=(c == DC - 1))

                # ---- PE-transpose x subtiles; copy psum->SBUF ----
                xts = []
                for jj in range(NSUB // 2):
                    psum_xt = ps_xt.tile([128, 2, DC, 128], BF16)
                    for h in range(2):
                        j = jj * 2 + h
                        js = slice(j * 128, (j + 1) * 128)
                        for c in range(DC):
                            nc.tensor.transpose(
                                psum_xt[:, h, c, :], xh[:, c, js], idbf_sb[:])
                    xt = xtpool.tile([128, 2, DC, 128], BF16, tag="xt")
                    if jj % 2 == 0:
                        nc.scalar.activation(xt[:], psum_xt[:], AF.Copy)
                    else:
                        nc.vector.tensor_copy(xt[:], psum_xt[:])
                    xts.append(xt)

                # ---- softmax over k (free axis), 8 subtiles at once ----
                x2b = x2_sb[:, b, g * NSUB:(g + 1) * NSUB, None] \
                    .broadcast_to([128, NSUB, K])
                s2db = s2d_sb[:, None, :].broadcast_to([128, NSUB, K])
                t1 = smpool.tile([128, NSUB, K], F32, tag="t1")
                nc.vector.tensor_tensor(t1[:], x2b, s2db, ALU.mult)
                es = smpool.tile([128, NSUB, K], F32, tag="es")
                nc.vector.tensor_tensor(es[:], psum_lin[:], t1[:], ALU.add)
                p = smpool.tile([128, NSUB, K], F32, tag="p")
                nc.scalar.activation(p[:], es[:], AF.Exp)
                s = smpool.tile([128, NSUB], F32, tag="s")
                nc.vector.tensor_reduce(s[:], p[:], AX.X, ALU.add)
                rec = smpool.tile([128, NSUB], F32, tag="rec")
                nc.vector.reciprocal(rec[:], s[:])
                a = smpool.tile([128, NSUB, K], BF16, tag="a")
                recb = rec[:, :, None].broadcast_to([128, NSUB, K])
                nc.vector.tensor_tensor(a[:], p[:], recb, ALU.mult)

                # ---- mm2/asum, accumulated over the whole batch ----
                for j in range(NSUB):
                    first = (g == 0 and j == 0)
                    last = (g == NG - 1 and j == NSUB - 1)
                    nc.tensor.matmul(
                        psum_as[:], lhsT=a[:, j, :], rhs=onbf_sb[:],
                        start=first, stop=last, skip_group_check=True)
                    nc.tensor.matmul(
                        psum_e[:], lhsT=a[:, j, :], rhs=xts[j // 2][:, j % 2],
                        start=first, stop=last, skip_group_check=True)

            # ---- e = psum_e - asum * c ----
            asb = psum_as[:, 0:1].broadcast_to([K, D])
            tmp = outpool.tile([K, D], F32, tag="tmp")
            nc.vector.tensor_tensor(tmp[:], asb, ckd_sb[:], ALU.mult)
            e_sb = outpool.tile([K, D], F32, tag="e_sb")
            nc.vector.tensor_tensor(e_sb[:], psum_e[:], tmp[:], ALU.subtract)
            nc.sync.dma_start(out=e[b], in_=e_sb[:])

    nc.compile()
    return nc


_NC_CACHE = None


def get_nc() -> bass.Bass:
    global _NC_CACHE
    if _NC_CACHE is None:
        _NC_CACHE = build_nc()
    return _NC_CACHE


def make_in_maps(x, codewords, scale):
    import ml_dtypes
    assert x.shape == (B, D, H, W) and codewords.shape == (K, D)
    x = np.ascontiguousarray(x, dtype=np.float32).reshape(B, D, N)
    codewords = np.ascontiguousarray(codewords, dtype=np.float32)
    scale = np.ascontiguousarray(scale, dtype=np.float32)

    x2 = (x.astype(np.float64) ** 2).sum(axis=1).astype(np.float32)  # [B, N]
    # pixel-major: x2sT[b, p, s] = x2[b, s*128 + p]
    x2sT = np.ascontiguousarray(x2.reshape(B, NS, 128).transpose(0, 2, 1))
    s2 = scale * scale                                   # [K]
    s2d = s2 - s2.max()
    s2drep = np.broadcast_to(s2d, (128, K)).copy()
    # cbf[dd, c, k] = -2*s2[k]*codewords[k, c*128+dd]
    cts = (-2.0 * s2[:, None] * codewords).T             # [D, K]
    cbf = np.ascontiguousarray(
        cts.reshape(DC, 128, K).transpose(1, 0, 2)).astype(ml_dtypes.bfloat16)
    ones_bf = np.ones((128, 2), ml_dtypes.bfloat16)
    ident_bf = np.eye(128, dtype=ml_dtypes.bfloat16)

    in_maps = []
    for i in range(NCORES):
        in_maps.append({
            "x": np.ascontiguousarray(x[i * BPC:(i + 1) * BPC]),
            "cbf": cbf, "c_kd": codewords, "s2drep": s2drep,
            "x2sT": np.ascontiguousarray(x2sT[i * BPC:(i + 1) * BPC]),
            "ones_bf": ones_bf, "ident_bf": ident_bf,
        })
    return in_maps


def kernel(x: np.ndarray, codewords: np.ndarray, scale: np.ndarray) -> np.ndarray:
    from concourse.bass_utils import run_bass_kernel_spmd

    in_maps = make_in_maps(x, codewords, scale)
    res = run_bass_kernel_spmd(get_nc(), in_maps, list(range(NCORES)))
    return np.concatenate([res.results[i]["e"] for i in range(NCORES)], axis=0)


# revision 4
# speedup vs baseline: 1.2514x; 1.2514x over previous
"""Trainium2 Bass kernel for nn_Encoding (VQ codebook soft-assignment encoding).

Reference computation (per batch b, with n = H*W pixels):
    xr[n, d]   = x[b].reshape(D, N).T
    sl[n, k]   = scale_k^2 * (||xr_n||^2 - 2 xr_n.c_k + ||c_k||^2)
    a[n, k]    = softmax_k(sl)
    e[b, k, d] = sum_n a[n,k] * xr[n,d]  -  (sum_n a[n,k]) * c[k,d]

Sharding: data-parallel over batch: 16 batches -> 8 cores x 2 batches each.
Codewords/scale replicated; no collectives.

Device mapping per core (B_PER_CORE=2, D=512, N=4096, K=32), processed in
groups of 1024 pixels (8 subtiles of 128 pixels):
  - x arrives [d, n] f32; DVE casts to bf16 once (2x mode).
  - mm1 runs with the x-tile as the STATIONARY operand (bf16 -> fast weight
    load) and the tiny codebook as moving: psum_lin[n128, k] += xh[d128,
    n128].T @ cbf[d128, k] accumulated over 4 d-chunks.  This lands the
    logits directly in [pixel, k] layout -- no logit transpose needed.
  - softmax shortcut (validated to 4e-8 frobenius vs exact): instead of a
    per-pixel max, subtract the bound x2_n*s2max:
        es = -2 s2_k (x.c_k) + x2_n (s2_k - s2max)   in [-900, ~1]
    The s2_k c2_k term is dropped entirely (|c2*s2| <= 1e-2, contributes
    ~2e-9 relative error).  x2 is precomputed on the host and shipped
    pre-transposed to pixel-major layout so it DMAs as contiguous lines.
  - x bf16 tiles are PE-transposed [128, 128] into psum, copied to SBUF
    (split across ACT and DVE), then mm2 contracts n:
    psum_e[k, d] += a[n128, k].T @ xt[n128, d512]; asum via a ones matmul
    in a parallel psum bank.
  - e = psum_e - asum[k]*c[k, d] on DVE, then DMA out.
"""

import numpy as np

import concourse.bass as bass
import concourse.bacc as bacc
import concourse.mybir as mybir
from concourse import tile

F32 = mybir.dt.float32
BF16 = mybir.dt.bfloat16
AF = mybir.ActivationFunctionType
AX = mybir.AxisListType
ALU = mybir.AluOpType

B, D, H, W, K = 16, 512, 64, 64, 32
N = H * W                    # 4096 pixels per batch
NCORES = 8
BPC = B // NCORES            # 2 batches per core
DC = D // 128                # 4 contraction chunks
NG = N // 1024               # 4 pixel-groups of 1024 per batch
NSUB = 8                     # 128-pixel subtiles per group
NS = N // 128                # 32 subtiles per batch


def build_nc() -> bass.Bass:
    nc = bacc.Bacc("TRN2", target_bir_lowering=False, debug=False,
                   num_devices=NCORES)

    x = nc.dram_tensor("x", [BPC, D, N], F32, kind="ExternalInput").ap()
    cbf = nc.dram_tensor("cbf", [128, DC, K], BF16, kind="ExternalInput").ap()
    c_kd = nc.dram_tensor("c_kd", [K, D], F32, kind="ExternalInput").ap()
    s2drep = nc.dram_tensor("s2drep", [128, K], F32, kind="ExternalInput").ap()
    x2sT = nc.dram_tensor("x2sT", [BPC, 128, NS], F32, kind="ExternalInput").ap()
    ones_bf = nc.dram_tensor("ones_bf", [128, 2], BF16, kind="ExternalInput").ap()
    ident_bf = nc.dram_tensor("ident_bf", [128, 128], BF16, kind="ExternalInput").ap()
    e = nc.dram_tensor("e", [BPC, K, D], F32, kind="ExternalOutput").ap()

    from contextlib import ExitStack
    with tile.TileContext(nc) as tc, ExitStack() as ctx:
        const = ctx.enter_context(tc.tile_pool(name="const", bufs=1))
        xpool = ctx.enter_context(tc.tile_pool(name="x", bufs=3))
        xhpool = ctx.enter_context(tc.tile_pool(name="xh", bufs=3))
        xtpool = ctx.enter_context(tc.tile_pool(name="xt", bufs=8))
        smpool = ctx.enter_context(tc.tile_pool(name="softmax", bufs=2))
        outpool = ctx.enter_context(tc.tile_pool(name="out", bufs=2))
        ps_lin = ctx.enter_context(tc.tile_pool(name="ps_lin", bufs=2, space="PSUM"))
        ps_xt = ctx.enter_context(tc.tile_pool(name="ps_xt", bufs=4, space="PSUM"))
        ps_e = ctx.enter_context(tc.tile_pool(name="ps_e", bufs=1, space="PSUM"))
        ps_as = ctx.enter_context(tc.tile_pool(name="ps_as", bufs=1, space="PSUM"))

        # Constants, loaded once.
        cbf_sb = const.tile([128, DC, K], BF16)
        nc.sync.dma_start(out=cbf_sb[:], in_=cbf[:])
        ckd_sb = const.tile([K, D], F32)
        nc.sync.dma_start(out=ckd_sb[:], in_=c_kd[:])
        s2d_sb = const.tile([128, K], F32)
        nc.sync.dma_start(out=s2d_sb[:], in_=s2drep[:])
        x2_sb = const.tile([128, BPC, NS], F32)
        for b in range(BPC):
            nc.sync.dma_start(out=x2_sb[:, b, :], in_=x2sT[b])
        onbf_sb = const.tile([128, 2], BF16)
        nc.sync.dma_start(out=onbf_sb[:], in_=ones_bf[:])
        idbf_sb = const.tile([128, 128], BF16)
        nc.sync.dma_start(out=idbf_sb[:], in_=ident_bf[:])

        for b in range(BPC):
            psum_e = ps_e.tile([K, D], F32)
            psum_as = ps_as.tile([K, 2], F32)
            for g in range(NG):
                n0 = g * 1024

                # ---- load one n-group of x: [128, DC, 1024] ([d, n]) ----
                xg = xpool.tile([128, DC, 1024], F32, tag="xg")
                for c in range(DC):
                    nc.sync.dma_start(
                        out=xg[:, c, :],
                        in_=x[b, c * 128:(c + 1) * 128, n0:n0 + 1024])

                # ---- cast to bf16 (DVE 2x), split per chunk ----
                xh = xhpool.tile([128, DC, 1024], BF16, tag="xh")
                for c in range(DC):
                    nc.vector.tensor_copy(xh[:, c, :], xg[:, c, :])

                # ---- transposes + copies + mm1, interleaved per pair ----
                psum_lin = ps_lin.tile([128, NSUB, K], F32)
                xts = []
                for jj in range(NSUB // 2):
                    psum_xt = ps_xt.tile([128, 2, DC, 128], BF16)
                    for h in range(2):
                        j = jj * 2 + h
                        js = slice(j * 128, (j + 1) * 128)
                        for c in range(DC):
                            nc.tensor.transpose(
                                psum_xt[:, h, c, :], xh[:, c, js], idbf_sb[:])
                    xt = xtpool.tile([128, 2, DC, 128], BF16, tag="xt")
                    if jj % 2 == 0:
                        nc.scalar.activation(xt[:], psum_xt[:], AF.Copy)
                    else:
                        nc.vector.tensor_copy(xt[:], psum_xt[:])
                    xts.append(xt)
                    # mm1 for the same two subtiles ([pixel, k] logits)
                    for h in range(2):
                        j = jj * 2 + h
                        js = slice(j * 128, (j + 1) * 128)
                        for c in range(DC):
                            nc.tensor.matmul(
                                psum_lin[:, j, :], lhsT=xh[:, c, js],
                                rhs=cbf_sb[:, c, :],
                                start=(c == 0), stop=(c == DC - 1))

                # ---- softmax over k (free axis), 8 subtiles at once ----
                x2b = x2_sb[:, b, g * NSUB:(g + 1) * NSUB, None] \
                    .broadcast_to([128, NSUB, K])
                s2db = s2d_sb[:, None, :].broadcast_to([128, NSUB, K])
                t1 = smpool.tile([128, NSUB, K], F32, tag="t1")
                nc.vector.tensor_tensor(t1[:], x2b, s2db, ALU.mult)
                es = smpool.tile([128, NSUB, K], F32, tag="es")
                nc.vector.tensor_tensor(es[:], psum_lin[:], t1[:], ALU.add)
                p = smpool.tile([128, NSUB, K], F32, tag="p")
                nc.scalar.activation(p[:], es[:], AF.Exp)
                s = smpool.tile([128, NSUB], F32, tag="s")
                nc.vector.tensor_reduce(s[:], p[:], AX.X, ALU.add)
                rec = smpool.tile([128, NSUB], F32, tag="rec")
                nc.vector.reciprocal(rec[:], s[:])
                a = smpool.tile([128, NSUB, K], BF16, tag="a")
                recb = rec[:, :, None].broadcast_to([128, NSUB, K])
                nc.vector.tensor_tensor(a[:], p[:], recb, ALU.mult)

                # ---- mm2/asum, accumulated over the whole batch ----
                for j in range(NSUB):
                    first = (g == 0 and j == 0)
                    last = (g == NG - 1 and j == NSUB - 1)
                    nc.tensor.matmul(
                        psum_as[:], lhsT=a[:, j, :], rhs=onbf_sb[:],
                        start=first, stop=last, skip_group_check=True)
                    nc.tensor.matmul(
                        psum_e[:], lhsT=a[:, j, :], rhs=xts[j // 2][:, j % 2],
                        start=first, stop=last, skip_group_check=True)

            # ---- e = psum_e - asum * c ----
            asb = psum_as[:, 0:1].broadcast_to([K, D])
            tmp = outpool.tile([K, D], F32, tag="tmp")
            nc.vector.tensor_tensor(tmp[:], asb, ckd_sb[:], ALU.mult)
            e_sb = outpool.tile([K, D], F32, tag="e_sb")
            nc.vector.tensor_tensor(e_sb[:], psum_e[:], tmp[:], ALU.subtract)
            nc.sync.dma_start(out=e[b], in_=e_sb[:])

    nc.compile()
    return nc


_NC_CACHE = None


def get_nc() -> bass.Bass:
    global _NC_CACHE
    if _NC_CACHE is None:
        _NC_CACHE = build_nc()
    return _NC_CACHE


def make_in_maps(x, codewords, scale):
    import ml_dtypes
    assert x.shape == (B, D, H, W) and codewords.shape == (K, D)
    x = np.ascontiguousarray(x, dtype=np.float32).reshape(B, D, N)
    codewords = np.ascontiguousarray(codewords, dtype=np.float32)
    scale = np.ascontiguousarray(scale, dtype=np.float32)

    x2 = (x.astype(np.float64) ** 2).sum(axis=1).astype(np.float32)  # [B, N]
    # pixel-major: x2sT[b, p, s] = x2[b, s*128 + p]
    x2sT = np.ascontiguousarray(x2.reshape(B, NS, 128).transpose(0, 2, 1))
    s2 = scale * scale                                   # [K]
    s2d = s2 - s2.max()
    s2drep = np.broadcast_to(s2d, (128, K)).copy()
    # cbf[dd, c, k] = -2*s2[k]*codewords[k, c*128+dd]
    cts = (-2.0 * s2[:, None] * codewords).T             # [D, K]
    cbf = np.ascontiguousarray(
        cts.reshape(DC, 128, K).transpose(1, 0, 2)).astype(ml_dtypes.bfloat16)
    ones_bf = np.ones((128, 2), ml_dtypes.bfloat16)
    ident_bf = np.eye(128, dtype=ml_dtypes.bfloat16)

    in_maps = []
    for i in range(NCORES):
        in_maps.append({
            "x": np.ascontiguousarray(x[i * BPC:(i + 1) * BPC]),
            "cbf": cbf, "c_kd": codewords, "s2drep": s2drep,
            "x2sT": np.ascontiguousarray(x2sT[i * BPC:(i + 1) * BPC]),
            "ones_bf": ones_bf, "ident_bf": ident_bf,
        })
    return in_maps


def kernel(x: np.ndarray, codewords: np.ndarray, scale: np.ndarray) -> np.ndarray:
    from concourse.bass_utils import run_bass_kernel_spmd

    in_maps = make_in_maps(x, codewords, scale)
    res = run_bass_kernel_spmd(get_nc(), in_maps, list(range(NCORES)))
    return np.concatenate([res.results[i]["e"] for i in range(NCORES)], axis=0)
